# revision 13
# baseline (speedup 1.0000x reference)
"""Trainium2 Bass kernel for the Dempster-Shafer sequential-combination layer.

Math (per batch element; inputs m[p, k], p=0..63 prototypes, k=0..10 with
slot 10 = omega):
    The reference left-fold is  M' = M*(m + w) + M_w*m  applied uniformly to
    all 11 slots (the omega slot picks up a 3x factor per step), followed by
    per-step normalization.  Normalization is a uniform positive scale and
    the step map is linear in M, so intermediate normalizations cancel in
    the final one.  Rescaling the state by the running omega product
    (y = M / M_w) turns the fold into
        y' = (Q + 1/3) * y + Q,      Q[p, k] = m[p, k] / (3 * w[p])
    with y_0 = m[0]/w[0] and y_omega == 1 identically, so
        out_k = y_k(63) / (1 + sum_{k<10} y_k(63)),   out_omega = 1 / (...)
    and neither the omega product nor the omega chains need computing.  This
    maps onto the DVE tensor_tensor_scan instruction (state = d0*state + d1)
    with per-(batch,k) chains laid p-contiguous along the free dimension; a
    zero in d0 at each chain start resets the recurrence across chains.

v2 engine choreography per chunk (vs the 442.5us v1 baseline):
  - the transposing chain-layout build moves OFF the DVE onto the Scalar
    engine as a Copy activation with strided writes, fused with the f32->bf16
    downconvert and the /3 scale;
  - 1/(w+delta) is the single-instruction DVE RECIPROCAL_APPROX_FAST custom
    op (51 ULP) instead of exp(-ln(3w)) on Scalar: the Scalar engine then
    only ever runs Copy, which lives in every activation table set, so the
    2/chunk ACT_TABLE_LOAD swaps (2.6us/chunk) disappear;
  - Q = mT * u runs on DVE as an all-bf16 step-1 tensor_tensor, which is
    eligible for the 2x_1P perf mode (2 elem/cyc);
  - d0 = Q + 1/3 runs as tensor_scalar on GpSimd/Scalar (split knob);
    chain-start zeroing on GpSimd;
  - the scan (bf16 in, fp32 state+out) and the tiny epilogue run on DVE.
bf16 quantization of m/u/Q/d0 gives norm-rel-err ~4.5e-3 (numpy-emulated),
well inside the 2e-2 gate; the scan state itself stays fp32.
"""

import numpy as np

B = 262144
P = 64
K = 11
KC = K - 1             # chains per group actually scanned (omega chain == 1)
N_CORES = 8
B_CORE = B // N_CORES  # 32768
NB = 8                 # batch groups per partition per chunk
DELTA = 1e-12          # guards w == 0 (present in uniform data)
THIRD = float(np.float32(1.0) / np.float32(3.0))
BUFS = 4
TSPLIT = 8             # groups 0..TSPLIT-1 of the transpose on Scalar, rest GpSimd
D0SPLIT = 2            # groups 0..D0SPLIT-1 of d0=Q+1/3 on Scalar, rest GpSimd
QSPLIT = 6             # groups 0..QSPLIT-1 of Q=mT*u on DVE, rest GpSimd

_CACHE = {}


def _build_program(reps=1, nb=NB, bufs=BUFS, tsplit=TSPLIT, d0split=D0SPLIT,
                   qsplit=QSPLIT, qdt="bf16", compute="full", pipeline="v2"):
    if pipeline == "v1":
        import kernel_v1_backup as v1
        return v1._build_program(reps=reps, nb=nb, bufs=bufs)

    import concourse.bacc as bacc
    import concourse.mybir as mybir
    from concourse.tile import TileContext

    f32 = mybir.dt.float32
    bt = mybir.dt.bfloat16 if qdt == "bf16" else mybir.dt.float32
    Alu = mybir.AluOpType
    Act = mybir.ActivationFunctionType

    n_chunks = B_CORE // (128 * nb)
    nc = bacc.Bacc(
        "TRN2", target_bir_lowering=False, debug=False, num_devices=N_CORES
    )
    x = nc.declare_dram_parameter("x", [B_CORE, P * K], f32, isOutput=False)
    out = nc.declare_dram_parameter("out", [B_CORE, K], f32, isOutput=True)

    xv = x.rearrange("(c i g) d -> c i (g d)", i=128, g=nb)
    ov = out.rearrange("(c i g) d -> c i (g d)", i=128, g=nb)

    with TileContext(nc) as tc:
        with tc.tile_pool(name="p", bufs=bufs) as pool:
            for _rep in range(reps):
                for c in range(n_chunks):
                    m_ = pool.tile([128, nb * P * K], f32, name="m_")
                    t_ = pool.tile([128, nb * P], f32, name="t_")
                    u_ = pool.tile([128, nb * P], bt, name="u_")
                    mt_ = pool.tile([128, nb * KC * P], bt, name="mt_")
                    d0_ = pool.tile([128, nb * KC * P], bt, name="d0_")
                    s_ = pool.tile([128, nb], f32, name="s_")
                    r_ = pool.tile([128, nb], f32, name="r_")
                    o_ = pool.tile([128, nb * K], f32, name="o_")

                    nc.sync.dma_start(out=m_[:], in_=xv[c])
                    if compute == "dmaonly":
                        nc.vector.tensor_copy(out=o_[:], in_=m_[:, : nb * K])
                        nc.sync.dma_start(out=ov[c], in_=o_[:])
                        continue

                    m4 = m_.rearrange("p (g q k) -> p g q k", g=nb, q=P, k=K)
                    t3 = t_.rearrange("p (g q) -> p g q", g=nb)

                    # t = w + delta (p>=1); t = (w + delta)/3 at chain starts
                    # (u then carries the 3x so Q_0 = m_0/w_0 after the /3
                    # folded into the transpose scale).
                    nc.scalar.activation(
                        out=t3[:, :, 1:], in_=m4[:, :, 1:, K - 1],
                        func=Act.Copy, bias=DELTA, scale=1.0,
                    )
                    nc.scalar.activation(
                        out=t3[:, :, 0:1], in_=m4[:, :, 0:1, K - 1],
                        func=Act.Copy, bias=DELTA / 3.0, scale=1.0 / 3.0,
                    )
                    # u = 1/t on DVE (custom op, fp32-only, in place), then
                    # downconvert to bf16 on Scalar
                    nc.vector.reciprocal_approx_fast(out=t_[:], in_=t_[:])
                    nc.scalar.activation(
                        out=u_[:], in_=t_[:], func=Act.Copy, bias=0.0,
                        scale=1.0,
                    )

                    # Transposing chain-layout build: mT[g, k, q] = m[g, q, k]/3
                    # (contiguous reads, strided writes, fused bf16 convert),
                    # split Scalar / GpSimd by group.
                    mt4 = mt_.rearrange("p (g k q) -> p g k q", g=nb, k=KC, q=P)
                    mtw = mt4.transpose([0, 1, 3, 2])   # [128, g, q, k]
                    mr = m4[:, :, :, :KC]
                    ts = max(0, min(nb, tsplit))
                    if ts > 0:
                        nc.scalar.activation(
                            out=mtw[:, :ts], in_=mr[:, :ts],
                            func=Act.Copy, bias=0.0, scale=1.0 / 3.0,
                        )
                    if ts < nb:
                        nc.gpsimd.tensor_scalar_mul(
                            out=mtw[:, ts:], in0=mr[:, ts:], scalar1=1.0 / 3.0,
                        )

                    # Q[g, k, q] = mT[g, k, q] * u[g, q]  (all-bf16, step-1 ->
                    # 2x_1P eligible on DVE), in place: mt_ holds Q afterwards;
                    # split DVE / GpSimd by group
                    q_ = mt_
                    q4 = q_.rearrange("p (g k q) -> p g k q", g=nb, k=KC, q=P)
                    ub = (
                        u_.rearrange("p (g q) -> p g q", g=nb)
                        .unsqueeze(2)
                        .broadcast_to([128, nb, KC, P])
                    )
                    qs = max(0, min(nb, qsplit))
                    if qs > 0:
                        nc.vector.tensor_tensor(
                            out=q4[:, :qs], in0=mt4[:, :qs], in1=ub[:, :qs],
                            op=Alu.mult,
                        )
                    if qs < nb:
                        nc.gpsimd.tensor_tensor(
                            out=q4[:, qs:], in0=mt4[:, qs:], in1=ub[:, qs:],
                            op=Alu.mult,
                        )

                    # d0 = Q + 1/3, split Scalar / GpSimd by group
                    d04 = d0_.rearrange("p (g k q) -> p g k q", g=nb, k=KC, q=P)
                    ds = max(0, min(nb, d0split))
                    if ds > 0:
                        nc.scalar.activation(
                            out=d04[:, :ds], in_=q4[:, :ds],
                            func=Act.Copy, bias=THIRD, scale=1.0,
                        )
                    if ds < nb:
                        nc.gpsimd.tensor_scalar_add(
                            out=d04[:, ds:], in0=q4[:, ds:], scalar1=THIRD,
                        )
                    # chain-start reset: d0 = 0 at q == 0
                    nc.gpsimd.memset(d04[:, :, :, 0:1], 0.0)

                    # y[t] = d0[t] * y[t-1] + Q[t]   (fp32 state, bf16 out,
                    # in place over d0_)
                    nc.vector.tensor_tensor_scan(
                        out=d0_[:], data0=d0_[:], data1=q_[:],
                        initial=0.0, op0=Alu.mult, op1=Alu.add,
                    )

                    y4 = d04
                    fin = y4[:, :, :, P - 1]  # [128, nb, KC]
                    nc.vector.tensor_reduce(
                        out=s_[:], in_=fin, axis=mybir.AxisListType.X,
                        op=Alu.add,
                    )
                    # r = 1 / (S + 1)   (the omega chain contributes exactly 1)
                    nc.vector.tensor_scalar(
                        out=s_[:], in0=s_[:], scalar1=1.0, scalar2=None,
                        op0=Alu.add,
                    )
                    nc.vector.reciprocal(out=r_[:], in_=s_[:])
                    rb = (
                        r_.rearrange("p g -> p g")
                        .unsqueeze(2)
                        .broadcast_to([128, nb, KC])
                    )
                    o3 = o_.rearrange("p (g k) -> p g k", g=nb)
                    nc.vector.tensor_tensor(
                        out=o3[:, :, :KC], in0=fin, in1=rb, op=Alu.mult
                    )
                    nc.vector.tensor_copy(
                        out=o3[:, :, KC:], in_=r_[:].unsqueeze(2)
                    )

                    # out-DMA triggered from the Pool queue so the SP queue
                    # carries only the input stream
                    nc.gpsimd.dma_start(out=ov[c], in_=o_[:])

    nc.compile()
    return nc


def kernel(inputs: np.ndarray) -> np.ndarray:
    from concourse.bass_utils import run_bass_kernel_spmd

    if "nc" not in _CACHE:
        _CACHE["nc"] = _build_program()
    nc = _CACHE["nc"]

    x = np.ascontiguousarray(np.asarray(inputs, dtype=np.float32)).reshape(
        B, P * K
    )
    shards = x.reshape(N_CORES, B_CORE, P * K)
    in_maps = [{"x": shards[i]} for i in range(N_CORES)]
    res = run_bass_kernel_spmd(nc, in_maps, core_ids=list(range(N_CORES)))
    outs = [res.results[i]["out"] for i in range(N_CORES)]
    return np.concatenate(outs, axis=0).reshape(B, K)
